# revision 46
# baseline (speedup 1.0000x reference)
"""Trainium2 Bass kernel for nn_CausalSelfAttention_6442450944521.

Sparse-attention causal self-attention block:
  B=4, T=2048 (rows<512: full attention over cols<512; rows>=512: causal),
  E=1024, H=16, D=64.

Sharding: batch (4) x head-group (2 groups of 8 heads) across 8 cores.
Each core computes qkv^T projections, block-sparse attention via S^T = K Q^T
tiles, and its row-slice of the output projection; the two head-group
partials per batch are summed on the host (row-parallel tensor parallelism).

Structure (measured ~315us vs the 354us padded-matmul baseline on the
same hardware):
  - S^T matmuls are ROW-TILED: the two heads of a pair run concurrently on
    the 128x128 PE array (head A rows 0-63 via tile_position (0,0), head B
    rows 64-127 via (64,0)) since the contraction dim is only D=64. kT/qT
    are stored pair-stacked [128, T] so base-partition slicing infers the
    tile positions; the paired matmul's slice shows ~4ns in traces (true
    concurrency), halving S wall time. No zero padding or memsets needed.
  - PV uses a full 128-col stationary [V|ones|zeros] (FWL-eligible; the
    ones column yields the softmax denominator for free); pad fills run
    during the input-DMA dead time at program start.
  - S blocks land in [128, 1024] two-bank PSUM bins; ONE exp per bin per
    head on the scalar engine (320 -> 160 ACT ops; per-op ACT overhead was
    pacing the attention phase).
  - Diagonal-block masks apply via one host-packed [128, 1280] mask tile.
  - Q/K bias adds + PSUM evacuations run on the vector engine; softmax
    normalization multiplies on gpsimd; denominator row -> DRAM ->
    partition-broadcast (gpsimd DMA queue) for pairs 0-2, and a K=1 PE
    broadcast matmul short-latency path for pair 3 (the endgame).
  - Projection (V/QK/out-proj) matmul units are interleaved between
    attention bins in PE program order so the PE fills scalar-exp waits;
    the placement is tuned so no region starves (PE micro-idle lets the
    HAM clock gate re-throttle the PE to 1.2 GHz -- cold 427ns matmuls).
  - Pair 3 runs q-groups descending so each finished q-group's output
    projections fill the next unit; tail is only qg0's projections.
  - Input-load dispatch (~0.6us per dma_start per sequencer) is spread
    across sync/scalar/gpsimd queues; only what the first ~25us needs is
    loaded early (x ch0-1, wv, low halves of wq/wk), wp deferred.
"""

import os
import sys

if "/opt/trn_rl_repo" not in sys.path:
    sys.path.insert(0, "/opt/trn_rl_repo")

import numpy as np

# Problem constants (hardcoded per harness contract).
B = 4
T = 2048
E = 1024
H = 16
D = 64
NCORES = 8
HPC = H // 2          # heads per core = 8
ESL = HPC * D         # per-core E-slice = 512
P = 128               # SBUF/PSUM partitions
TG = 512              # q-group width
NTG = T // TG         # 4
NTT = T // P          # 16
NEC = E // P          # 8 contraction chunks over E
NPAIR = HPC // 2      # 4 head-pair tiles

_CACHE = {}


def _att_bins(qg):
    """Bins of S^T blocks for q-group qg. Each bin is a list of
    (kt, c0, s0, n): k-tile index, column offset in the [128,1024] bin,
    q-offset within the group, and width. Total bin width <= 1024."""
    bins = []
    nf = 4 if qg == 0 else 4 * qg
    for k0 in range(0, nf, 2):
        bins.append([(k0, 0, 0, TG), (k0 + 1, TG, 0, TG)])
    if qg > 0:
        m0 = 4 * qg
        bins.append([(m0, 0, 0, 512), (m0 + 1, 512, 128, 384)])
        bins.append([(m0 + 2, 0, 256, 256), (m0 + 3, 256, 384, 128)])
    return bins


def _build_program():
    import concourse.bass as bass
    import concourse.tile as tile
    from concourse import bacc, mybir

    f32 = mybir.dt.float32
    bf16 = mybir.dt.bfloat16

    nc = bacc.Bacc("TRN2", target_bir_lowering=False, debug=False,
                   num_devices=NCORES)

    xT = nc.dram_tensor("xT", [E, T], bf16, kind="ExternalInput").ap()
    wq = nc.dram_tensor("wq", [E, ESL], bf16, kind="ExternalInput").ap()
    wk = nc.dram_tensor("wk", [E, ESL], bf16, kind="ExternalInput").ap()
    wv = nc.dram_tensor("wv", [E, ESL], bf16, kind="ExternalInput").ap()
    wp = nc.dram_tensor("wp", [ESL, E], bf16, kind="ExternalInput").ap()
    bias = nc.dram_tensor("bias", [P, 2 * NPAIR], f32,
                          kind="ExternalInput").ap()
    binmask = nc.dram_tensor("binmask", [P, 1280], bf16,
                             kind="ExternalInput").ap()
    out = nc.dram_tensor("out", [T, E], f32, kind="ExternalOutput").ap()

    with tile.TileContext(nc) as tc:
        _body(nc, tc, tile, mybir, bass,
              xT, wq, wk, wv, wp, bias, binmask, out)

    nc.compile()
    return nc


def _body(nc, tc, tile, mybir, bass,
          xT, wq, wk, wv, wp, bias, binmask, out):
    f32 = mybir.dt.float32
    bf16 = mybir.dt.bfloat16
    Exp = mybir.ActivationFunctionType.Exp

    cms = {}

    def open_pool(name, bufs, space=None, side=None):
        kw = {}
        if space:
            kw["space"] = space
        if side:
            kw["side"] = side
        cm = tc.tile_pool(name=name, bufs=bufs, **kw)
        pool = cm.__enter__()
        cms[id(pool)] = cm
        return pool

    def close_pool(pool):
        cms.pop(id(pool)).__exit__(None, None, None)

    # ---- pools ----------------------------------------------------------
    singles = open_pool("singles", 1)
    ps = open_pool("ps", 2, space="PSUM")
    pT_pool = open_pool("pT", 5)
    ob_pool = open_pool("ob", 2)
    bc_pool = open_pool("bc", 2)
    on_pool = open_pool("on", 2)
    ot_pool = open_pool("ot", 4)
    dr_pool = open_pool("dr", 2, space="DRAM")
    # right-stack: big resident tensors
    res_pool = open_pool("res", 1, side="right")

    # ---- resident loads --------------------------------------------------
    mask_t = singles.tile([P, 1280], bf16, tag="mask", name="mask")
    nc.sync.dma_start(out=mask_t[:], in_=binmask)
    bias_t = singles.tile([P, 2 * NPAIR], f32, tag="bias", name="bias")
    nc.sync.dma_start(out=bias_t[:], in_=bias)

    # Input loads: dispatch is the startup bottleneck (~0.6us per dma_start
    # on one sequencer), so spread it across the three DMA-capable engines.
    # sync: x chunk 0 + wk + x chunk 1 + wp; scalar: wv + wq (idle until the
    # first exp anyway); gpsimd: x chunks 2-3.
    xr = [res_pool.tile([P, T], bf16, tag=f"xr{ec}", name=f"xr{ec}")
          for ec in range(NEC)]
    wq_c, wk_c, wv_c, wp_c = [], [], [], []
    for ec in range(NEC):
        wv_c.append(res_pool.tile([P, ESL], bf16, tag="wv", name="wvc",
                                  bufs=NEC))
        wq_c.append(res_pool.tile([P, ESL], bf16, tag="wq", name="wqc",
                                  bufs=NEC))
        wk_c.append(res_pool.tile([P, ESL], bf16, tag="wk", name="wkc",
                                  bufs=NEC))
    H2 = ESL // 2
    for ec in range(NEC):
        nc.sync.dma_start(out=xr[ec][:, 0:T // 4],
                          in_=xT[ec * P:(ec + 1) * P, 0:T // 4])
        nc.scalar.dma_start(out=wv_c[ec][:], in_=wv[ec * P:(ec + 1) * P, :])
        nc.gpsimd.dma_start(out=wk_c[ec][:, 0:H2],
                            in_=wk[ec * P:(ec + 1) * P, 0:H2])
    for ec in range(NEC):
        nc.sync.dma_start(out=xr[ec][:, T // 4:T // 2],
                          in_=xT[ec * P:(ec + 1) * P, T // 4:T // 2])
        nc.scalar.dma_start(out=wq_c[ec][:, 0:H2],
                            in_=wq[ec * P:(ec + 1) * P, 0:H2])
        nc.gpsimd.dma_start(out=xr[ec][:, T // 2:3 * T // 4],
                            in_=xT[ec * P:(ec + 1) * P, T // 2:3 * T // 4])
    for ec in range(NEC):
        nc.gpsimd.dma_start(out=xr[ec][:, 3 * T // 4:T],
                            in_=xT[ec * P:(ec + 1) * P, 3 * T // 4:T])
        nc.scalar.dma_start(out=wq_c[ec][:, H2:ESL],
                            in_=wq[ec * P:(ec + 1) * P, H2:ESL])
        nc.sync.dma_start(out=wk_c[ec][:, H2:ESL],
                          in_=wk[ec * P:(ec + 1) * P, H2:ESL])
    wp_c = [res_pool.tile([P, E], bf16, tag="wp", name="wpc", bufs=NPAIR)
            for c in range(NPAIR)]

    qT_t = [res_pool.tile([P, T], bf16, tag=f"qT{i}", name=f"qT{i}")
            for i in range(NPAIR)]
    kT_t = [res_pool.tile([P, T], bf16, tag=f"kT{i}", name=f"kT{i}")
            for i in range(NPAIR)]
    yT_t = [res_pool.tile([P, T], bf16, tag=f"yT{i}", name=f"yT{i}")
            for i in range(NPAIR)]
    # V per T-tile: per head [V(64) | ones | zeros(63)] = full 128-col
    # stationary (FWL-eligible). Zero/ones fills run during the input-DMA
    # dead time at program start.
    v_t = [res_pool.tile([P, HPC, P], bf16, tag=f"v{i}", name=f"v{i}")
           for i in range(NTT)]
    for tt in range(NTT):
        nc.vector.memset(v_t[tt][:, :, D + 1:], 0.0)
        nc.vector.memset(v_t[tt][:, :, D:D + 1], 1.0)

    # ---- filler units (PE work interleaved between attention bins) ------
    def emit_v(tt):
        ts_ = slice(tt * P, (tt + 1) * P)
        psv = ps.tile([P, ESL], f32, tag="mm", name="psv", bufs=2)
        for ec in range(NEC):
            nc.tensor.matmul(psv[:], lhsT=xr[ec][:, ts_], rhs=wv_c[ec][:],
                             start=(ec == 0), stop=(ec == NEC - 1))
        nc.vector.tensor_copy(v_t[tt][:, :, 0:D], psv[:])

    def emit_qk(pt, tg):
        cs = slice(tg * TG, (tg + 1) * TG)
        pcol = slice(pt * P, (pt + 1) * P)
        psq = ps.tile([P, TG], f32, tag="mm", name="psq", bufs=2)
        for ec in range(NEC):
            nc.tensor.matmul(psq[:], lhsT=wq_c[ec][:, pcol],
                             rhs=xr[ec][:, cs],
                             start=(ec == 0), stop=(ec == NEC - 1))
        nc.vector.tensor_scalar_add(qT_t[pt][:, cs], psq[:],
                                    bias_t[:, pt:pt + 1])
        psk = ps.tile([P, TG], f32, tag="mm", name="psk", bufs=2)
        for ec in range(NEC):
            nc.tensor.matmul(psk[:], lhsT=wk_c[ec][:, pcol],
                             rhs=xr[ec][:, cs],
                             start=(ec == 0), stop=(ec == NEC - 1))
        nc.vector.tensor_scalar_add(kT_t[pt][:, cs], psk[:],
                                    bias_t[:, NPAIR + pt:NPAIR + pt + 1])

    def emit_proj(tt, tail=False):
        ts_ = slice(tt * P, (tt + 1) * P)
        for ng in range(E // TG):
            pp = ps.tile([P, TG], f32, tag="mm", name="pp", bufs=2)
            for c in range(NPAIR):
                nc.tensor.matmul(pp[:], lhsT=yT_t[c][:, ts_],
                                 rhs=wp_c[c][:, ng * TG:(ng + 1) * TG],
                                 start=(c == 0), stop=(c == NPAIR - 1))
            ot = ot_pool.tile([P, TG], f32, tag="ot", name="ot")
            # the true tail runs after the last exp: the scalar engine is
            # idle there, so alternate evacuations across both engines
            if tail and ng == 1:
                nc.scalar.copy(ot[:], pp[:])
            else:
                nc.vector.tensor_copy(ot[:], pp[:])
            cs_ = slice(ng * TG, (ng + 1) * TG)
            nc.sync.dma_start(out=out[tt * P:tt * P + 64, cs_],
                              in_=ot[0:64, :])
            nc.sync.dma_start(out=out[tt * P + 64:(tt + 1) * P, cs_],
                              in_=ot[64:P, :])

    ones_t = singles.tile([P, D], bf16, tag="ones", name="ones")
    nc.vector.memset(ones_t[:], 1.0)

    # ---- attention for one (pair, q-group): both heads row-tiled ---------
    def emit_att(pt, qg, fillers, last=False):
        qb = qg * TG
        bins = _att_bins(qg)
        nbin = len(bins)
        pend = []  # (bin index, pT tiles per head, bin entries)
        blocks_total = sum(len(b) for b in bins)
        blk_idx = [0, 0]   # per-head running PV block index
        po = [ps.tile([P, TG], f32, tag="o", name=f"po{j}", bufs=2)
              for j in range(2)]

        def emit_pv(ent, pTs):
            for j in range(2):
                for (kt, c0, s0, n) in ent:
                    i = blk_idx[j]
                    nc.tensor.matmul(po[j][:, s0:TG],
                                     lhsT=v_t[kt][:, 2 * pt + j, :],
                                     rhs=pTs[j][:, c0:c0 + n],
                                     start=(i == 0),
                                     stop=(i == blocks_total - 1))
                    blk_idx[j] += 1

        fill_iter = iter(fillers)
        for bi, ent in enumerate(bins):
            width = sum(n for (_, _, _, n) in ent)
            diag = qg > 0 and bi >= nbin - 2
            pss = [ps.tile([P, 2 * TG], f32, tag="s", name=f"pss{j}", bufs=2)
                   for j in range(2)]
            pTs = [pT_pool.tile([P, 2 * TG], bf16, tag="pT", name=f"pT{j}")
                   for j in range(2)]
            # S^T row-tiled: head j on array rows 64j..64j+63.
            for (kt, c0, s0, n) in ent:
                ks = slice(kt * P, (kt + 1) * P)
                qs = slice(qb + s0, qb + TG)
                for j in range(2):
                    rw = slice(64 * j, 64 * (j + 1))
                    nc.tensor.matmul(pss[j][:, c0:c0 + n],
                                     lhsT=kT_t[pt][rw, ks],
                                     rhs=qT_t[pt][rw, qs],
                                     start=True, stop=True)
            for j in range(2):
                nc.scalar.activation(pTs[j][:, 0:width], pss[j][:, 0:width],
                                     Exp, scale=0.125)
                if diag:
                    moff = 0 if bi == nbin - 2 else 896
                    nc.vector.tensor_mul(pTs[j][:, 0:width],
                                         pTs[j][:, 0:width],
                                         mask_t[:, moff:moff + width])
            pend.append((pTs, ent))
            # PV lags S by one bin so the PE isn't waiting on exp.
            if len(pend) > 1:
                ppTs, pent = pend.pop(0)
                emit_pv(pent, ppTs)
            for f in fill_iter:
                f()
                break
        ppTs, pent = pend.pop(0)
        emit_pv(pent, ppTs)
        for f in fill_iter:
            f()

        # ---- normalization (off the PE path) ----
        on = on_pool.tile([D, 2, TG], bf16, tag="on", name="on")
        rc = bc_pool.tile([D, 2, TG], f32, tag="rc", name="rc")
        if last:
            # Short-latency tail path: broadcast the denominator row across
            # partitions with a K=1 PE matmul (PSUM is free by now), then
            # normalize straight out of PSUM.
            den_b = ob_pool.tile([D + 1, 2, TG], bf16, tag="db", name="db")
            for j in range(2):
                nc.vector.tensor_copy(den_b[D:D + 1, j, :],
                                      po[j][D:D + 1, :])
            psb = ps.tile([P, 2 * TG], f32, tag="s", name="psb", bufs=2)
            for j in range(2):
                nc.tensor.matmul(psb[0:D, j * TG:(j + 1) * TG],
                                 lhsT=ones_t[64:65, :],
                                 rhs=den_b[D:D + 1, j, :],
                                 start=True, stop=True)
            for j in range(2):
                nc.vector.reciprocal_approx_fast(
                    out=rc[:, j, :], in_=psb[0:D, j * TG:(j + 1) * TG])
                nc.vector.tensor_mul(on[:, j, :], po[j][0:D, :], rc[:, j, :])
        else:
            ob = ob_pool.tile([D + 1, 2, TG], f32, tag="ob", name="ob")
            for j in range(2):
                nc.vector.tensor_copy(ob[:, j, :], po[j][0:D + 1, :])
            den_d = dr_pool.tile([1, 2, TG], f32, tag="den", name="den")
            nc.gpsimd.dma_start(out=den_d[:], in_=ob[D:D + 1, :, :])
            bcast_in = bass.AP(
                tensor=den_d.tensor, offset=den_d.offset,
                ap=[[0, D]] + [list(a) for a in den_d.ap[1:]])
            bc = bc_pool.tile([D, 2, TG], f32, tag="bc", name="bc")
            nc.gpsimd.dma_start(out=bc[:], in_=bcast_in)
            nc.vector.reciprocal_approx_fast(out=rc[:], in_=bc[:])
            nc.gpsimd.tensor_mul(on[:], ob[0:D, :, :], rc[:])
        for j in range(2):
            nc.gpsimd.dma_start(
                out=yT_t[pt][64 * j:64 * (j + 1), qb:qb + TG],
                in_=on[:, j, :])

    # ---- schedule --------------------------------------------------------
    # Warm-up: V tiles 0-3 and pair-0 tg-0 QK, then attention with
    # remaining projection work interleaved as PE fillers.
    for tt in range(4):
        emit_v(tt)
    emit_qk(0, 0)
    # wp is first needed by out-proj at ~240us; load it after the startup
    # crunch so it doesn't steal early DMA bandwidth from x.
    for c in range(NPAIR):
        nc.sync.dma_start(out=wp_c[c][:], in_=wp[c * P:(c + 1) * P, :])

    fillers = {
        (0, 0): [lambda: emit_qk(0, 1)] +
                [lambda t=t: emit_v(t) for t in range(4, 8)],
        (0, 1): [lambda: emit_qk(0, 2)] +
                [lambda t=t: emit_v(t) for t in range(8, 12)],
        (0, 2): [lambda: emit_qk(0, 3)] +
                [lambda t=t: emit_v(t) for t in range(12, 16)],
        (0, 3): [lambda: emit_qk(1, 0), lambda: emit_qk(1, 1)],
        (1, 0): [lambda: emit_qk(1, 2)],
        (1, 1): [lambda: emit_qk(1, 3)],
        (1, 2): [lambda: emit_qk(2, 0)],
        (1, 3): [lambda: emit_qk(2, 1)],
        (2, 0): [lambda: emit_qk(2, 2), lambda: emit_qk(3, 0)],
        (2, 1): [lambda: emit_qk(2, 3), lambda: emit_qk(3, 1)],
        (2, 2): [lambda: emit_qk(3, 2)],
        (2, 3): [lambda: emit_qk(3, 3)],
        (3, 2): [lambda t=t: emit_proj(t) for t in range(12, 16)],
        (3, 1): [lambda t=t: emit_proj(t) for t in range(8, 12)],
        (3, 0): [lambda t=t: emit_proj(t) for t in range(4, 8)],
    }
    # Pair 3 runs q-groups descending so each completed q-group's output
    # projections fill the next (smaller) attention unit; the tail is only
    # qg 0's projections.
    qg_order = {3: [3, 2, 1, 0]}
    for pt in range(NPAIR):
        for qg in qg_order.get(pt, range(NTG)):
            emit_att(pt, qg, fillers.get((pt, qg), []), last=(pt == 3))
    for tt in range(0, 4):
        emit_proj(tt, tail=True)

    close_pool(res_pool)
    close_pool(dr_pool)
    close_pool(ot_pool)
    close_pool(on_pool)
    close_pool(bc_pool)
    close_pool(ob_pool)
    close_pool(pT_pool)
    close_pool(ps)
    close_pool(singles)


def _get_program():
    if "nc" not in _CACHE:
        _CACHE["nc"] = _build_program()
    return _CACHE["nc"]


def make_in_maps(x, W_qkv, b_qkv, W_proj):
    """Per-core input dicts: core c -> (batch c%4, head-group c//4)."""
    import ml_dtypes
    x = np.asarray(x, np.float32)
    W_qkv = np.asarray(W_qkv, np.float32)
    b_qkv = np.asarray(b_qkv, np.float32)
    # Packed diagonal-bin mask: segments tri(512)|tri(384)|tri(256)|tri(128);
    # tri(n)[p, j] = (j >= p) for j in [0, n).
    segs = [512, 384, 256, 128]
    binmask = np.zeros((P, sum(segs)), np.float32)
    off = 0
    for n in segs:
        binmask[:, off:off + n] = (np.arange(n)[None, :] >=
                                   np.arange(P)[:, None])
        off += n
    cvt = lambda a: np.ascontiguousarray(a).astype(ml_dtypes.bfloat16)
    in_maps = []
    for c in range(NCORES):
        b, g = c % B, c // B
        gs = slice(g * ESL, (g + 1) * ESL)
        bqs = b_qkv[0 * E:1 * E][gs]
        bks = b_qkv[1 * E:2 * E][gs]
        bias = np.zeros((P, 2 * NPAIR), np.float32)
        for pt in range(NPAIR):
            bias[:, pt] = bqs[pt * P:(pt + 1) * P]
            bias[:, NPAIR + pt] = bks[pt * P:(pt + 1) * P]
        in_maps.append({
            "xT": cvt(x[b].T),
            "wq": cvt(W_qkv[:, 0 * E:1 * E][:, gs]),
            "wk": cvt(W_qkv[:, 1 * E:2 * E][:, gs]),
            "wv": cvt(W_qkv[:, 2 * E:3 * E][:, gs]),
            "wp": cvt(np.asarray(W_proj, np.float32)[gs, :]),
            "bias": np.ascontiguousarray(bias),
            "binmask": cvt(binmask),
        })
    return in_maps


def gather_output(results, b_qkv, b_proj, W_proj):
    """Sum the two row-parallel partials per batch; fold v/proj biases."""
    b_qkv = np.asarray(b_qkv, np.float64)
    W_proj = np.asarray(W_proj, np.float64)
    b_v = b_qkv[2 * E:3 * E]
    const = b_v @ W_proj + np.asarray(b_proj, np.float64)
    out = np.empty((B, T, E), np.float32)
    for b in range(B):
        out[b] = (results[b]["out"].astype(np.float64) +
                  results[b + B]["out"].astype(np.float64) +
                  const).astype(np.float32)
    return out


def run_on_hw(inputs, trace=False, **kwargs):
    from concourse.bass_utils import run_bass_kernel_spmd
    nc = _get_program()
    in_maps = make_in_maps(inputs["x"], inputs["W_qkv"], inputs["b_qkv"],
                           inputs["W_proj"])
    res = run_bass_kernel_spmd(nc, in_maps, list(range(NCORES)), trace=trace,
                               **kwargs)
    out = gather_output(res.results, inputs["b_qkv"], inputs["b_proj"],
                        inputs["W_proj"])
    return out, res


def kernel(x, W_qkv, b_qkv, W_proj, b_proj):
    out, _ = run_on_hw({"x": x, "W_qkv": W_qkv, "b_qkv": b_qkv,
                        "W_proj": W_proj, "b_proj": b_proj})
    return out


# revision 49
# speedup vs baseline: 1.0013x; 1.0013x over previous
"""Trainium2 Bass kernel for nn_CausalSelfAttention_6442450944521.

Sparse-attention causal self-attention block:
  B=4, T=2048 (rows<512: full attention over cols<512; rows>=512: causal),
  E=1024, H=16, D=64.

Sharding: batch (4) x head-group (2 groups of 8 heads) across 8 cores.
Each core computes qkv^T projections, block-sparse attention via S^T = K Q^T
tiles, and its row-slice of the output projection; the two head-group
partials per batch are summed on the host (row-parallel tensor parallelism).

Structure (measured ~315us vs the 354us padded-matmul baseline on the
same hardware):
  - S^T matmuls are ROW-TILED: the two heads of a pair run concurrently on
    the 128x128 PE array (head A rows 0-63 via tile_position (0,0), head B
    rows 64-127 via (64,0)) since the contraction dim is only D=64. kT/qT
    are stored pair-stacked [128, T] so base-partition slicing infers the
    tile positions; the paired matmul's slice shows ~4ns in traces (true
    concurrency), halving S wall time. No zero padding or memsets needed.
  - PV uses a full 128-col stationary [V|ones|zeros] (FWL-eligible; the
    ones column yields the softmax denominator for free); pad fills run
    during the input-DMA dead time at program start.
  - S blocks land in [128, 1024] two-bank PSUM bins; ONE exp per bin per
    head on the scalar engine (320 -> 160 ACT ops; per-op ACT overhead was
    pacing the attention phase).
  - Diagonal-block masks apply via one host-packed [128, 1280] mask tile.
  - Q/K bias adds + PSUM evacuations run on the vector engine; softmax
    normalization multiplies on gpsimd; denominator row -> DRAM ->
    partition-broadcast (gpsimd DMA queue) for pairs 0-2, and a K=1 PE
    broadcast matmul short-latency path for pair 3 (the endgame).
  - Projection (V/QK/out-proj) matmul units are interleaved between
    attention bins in PE program order so the PE fills scalar-exp waits;
    the placement is tuned so no region starves (PE micro-idle lets the
    HAM clock gate re-throttle the PE to 1.2 GHz -- cold 427ns matmuls).
  - Pair 3 runs q-groups descending so each finished q-group's output
    projections fill the next unit; tail is only qg0's projections.
  - Input-load dispatch (~0.6us per dma_start per sequencer) is spread
    across sync/scalar/gpsimd queues; only what the first ~25us needs is
    loaded early (x ch0-1, wv, low halves of wq/wk), wp deferred.
"""

import os
import sys

if "/opt/trn_rl_repo" not in sys.path:
    sys.path.insert(0, "/opt/trn_rl_repo")

import numpy as np

# Problem constants (hardcoded per harness contract).
B = 4
T = 2048
E = 1024
H = 16
D = 64
NCORES = 8
HPC = H // 2          # heads per core = 8
ESL = HPC * D         # per-core E-slice = 512
P = 128               # SBUF/PSUM partitions
TG = 512              # q-group width
NTG = T // TG         # 4
NTT = T // P          # 16
NEC = E // P          # 8 contraction chunks over E
NPAIR = HPC // 2      # 4 head-pair tiles

_CACHE = {}


def _att_bins(qg):
    """Bins of S^T blocks for q-group qg. Each bin is a list of
    (kt, c0, s0, n): k-tile index, column offset in the [128,1024] bin,
    q-offset within the group, and width. Total bin width <= 1024."""
    bins = []
    nf = 4 if qg == 0 else 4 * qg
    for k0 in range(0, nf, 2):
        bins.append([(k0, 0, 0, TG), (k0 + 1, TG, 0, TG)])
    if qg > 0:
        m0 = 4 * qg
        bins.append([(m0, 0, 0, 512), (m0 + 1, 512, 128, 384)])
        bins.append([(m0 + 2, 0, 256, 256), (m0 + 3, 256, 384, 128)])
    return bins


def _build_program():
    import concourse.bass as bass
    import concourse.tile as tile
    from concourse import bacc, mybir

    f32 = mybir.dt.float32
    bf16 = mybir.dt.bfloat16

    nc = bacc.Bacc("TRN2", target_bir_lowering=False, debug=False,
                   num_devices=NCORES)

    xT = nc.dram_tensor("xT", [E, T], bf16, kind="ExternalInput").ap()
    wq = nc.dram_tensor("wq", [E, ESL], bf16, kind="ExternalInput").ap()
    wk = nc.dram_tensor("wk", [E, ESL], bf16, kind="ExternalInput").ap()
    wv = nc.dram_tensor("wv", [E, ESL], bf16, kind="ExternalInput").ap()
    wp = nc.dram_tensor("wp", [ESL, E], bf16, kind="ExternalInput").ap()
    bias = nc.dram_tensor("bias", [P, 2 * NPAIR], f32,
                          kind="ExternalInput").ap()
    binmask = nc.dram_tensor("binmask", [P, 1280], bf16,
                             kind="ExternalInput").ap()
    out = nc.dram_tensor("out", [T, E], f32, kind="ExternalOutput").ap()

    with tile.TileContext(nc) as tc:
        _body(nc, tc, tile, mybir, bass,
              xT, wq, wk, wv, wp, bias, binmask, out)

    nc.compile()
    return nc


def _body(nc, tc, tile, mybir, bass,
          xT, wq, wk, wv, wp, bias, binmask, out):
    f32 = mybir.dt.float32
    bf16 = mybir.dt.bfloat16
    Exp = mybir.ActivationFunctionType.Exp

    cms = {}

    def open_pool(name, bufs, space=None, side=None):
        kw = {}
        if space:
            kw["space"] = space
        if side:
            kw["side"] = side
        cm = tc.tile_pool(name=name, bufs=bufs, **kw)
        pool = cm.__enter__()
        cms[id(pool)] = cm
        return pool

    def close_pool(pool):
        cms.pop(id(pool)).__exit__(None, None, None)

    # ---- pools ----------------------------------------------------------
    singles = open_pool("singles", 1)
    ps = open_pool("ps", 2, space="PSUM")
    pT_pool = open_pool("pT", 5)
    ob_pool = open_pool("ob", 2)
    bc_pool = open_pool("bc", 2)
    on_pool = open_pool("on", 2)
    ot_pool = open_pool("ot", 4)
    dr_pool = open_pool("dr", 2, space="DRAM")
    # right-stack: big resident tensors
    res_pool = open_pool("res", 1, side="right")

    # ---- resident loads --------------------------------------------------
    mask_t = singles.tile([P, 1280], bf16, tag="mask", name="mask")
    nc.sync.dma_start(out=mask_t[:], in_=binmask)
    bias_t = singles.tile([P, 2 * NPAIR], f32, tag="bias", name="bias")
    nc.sync.dma_start(out=bias_t[:], in_=bias)

    # Input loads: dispatch is the startup bottleneck (~0.6us per dma_start
    # on one sequencer), so spread it across the three DMA-capable engines.
    # sync: x chunk 0 + wk + x chunk 1 + wp; scalar: wv + wq (idle until the
    # first exp anyway); gpsimd: x chunks 2-3.
    xr = [res_pool.tile([P, T], bf16, tag=f"xr{ec}", name=f"xr{ec}")
          for ec in range(NEC)]
    wq_c, wk_c, wv_c, wp_c = [], [], [], []
    for ec in range(NEC):
        wv_c.append(res_pool.tile([P, ESL], bf16, tag="wv", name="wvc",
                                  bufs=NEC))
        wq_c.append(res_pool.tile([P, ESL], bf16, tag="wq", name="wqc",
                                  bufs=NEC))
        wk_c.append(res_pool.tile([P, ESL], bf16, tag="wk", name="wkc",
                                  bufs=NEC))
    H2 = ESL // 2
    for ec in range(NEC):
        nc.sync.dma_start(out=xr[ec][:, 0:T // 4],
                          in_=xT[ec * P:(ec + 1) * P, 0:T // 4])
        nc.scalar.dma_start(out=wv_c[ec][:], in_=wv[ec * P:(ec + 1) * P, :])
        nc.gpsimd.dma_start(out=wk_c[ec][:, 0:H2],
                            in_=wk[ec * P:(ec + 1) * P, 0:H2])
    for ec in range(NEC):
        nc.sync.dma_start(out=xr[ec][:, T // 4:T // 2],
                          in_=xT[ec * P:(ec + 1) * P, T // 4:T // 2])
        nc.scalar.dma_start(out=wq_c[ec][:, 0:H2],
                            in_=wq[ec * P:(ec + 1) * P, 0:H2])
        nc.gpsimd.dma_start(out=xr[ec][:, T // 2:3 * T // 4],
                            in_=xT[ec * P:(ec + 1) * P, T // 2:3 * T // 4])
    for ec in range(NEC):
        nc.gpsimd.dma_start(out=xr[ec][:, 3 * T // 4:T],
                            in_=xT[ec * P:(ec + 1) * P, 3 * T // 4:T])
        nc.scalar.dma_start(out=wq_c[ec][:, H2:ESL],
                            in_=wq[ec * P:(ec + 1) * P, H2:ESL])
        nc.sync.dma_start(out=wk_c[ec][:, H2:ESL],
                          in_=wk[ec * P:(ec + 1) * P, H2:ESL])
    wp_c = [res_pool.tile([P, E], bf16, tag="wp", name="wpc", bufs=NPAIR)
            for c in range(NPAIR)]

    qT_t = [res_pool.tile([P, T], bf16, tag=f"qT{i}", name=f"qT{i}")
            for i in range(NPAIR)]
    kT_t = [res_pool.tile([P, T], bf16, tag=f"kT{i}", name=f"kT{i}")
            for i in range(NPAIR)]
    yT_t = [res_pool.tile([P, T], bf16, tag=f"yT{i}", name=f"yT{i}")
            for i in range(NPAIR)]
    # V per T-tile: per head [V(64) | ones | zeros(63)] = full 128-col
    # stationary (FWL-eligible). Zero/ones fills run during the input-DMA
    # dead time at program start.
    v_t = [res_pool.tile([P, HPC, P], bf16, tag=f"v{i}", name=f"v{i}")
           for i in range(NTT)]
    for tt in range(NTT):
        nc.vector.memset(v_t[tt][:, :, D + 1:], 0.0)
        nc.vector.memset(v_t[tt][:, :, D:D + 1], 1.0)

    # ---- filler units (PE work interleaved between attention bins) ------
    def emit_v(tt):
        ts_ = slice(tt * P, (tt + 1) * P)
        psv = ps.tile([P, ESL], f32, tag="mm", name="psv", bufs=2)
        for ec in range(NEC):
            nc.tensor.matmul(psv[:], lhsT=xr[ec][:, ts_], rhs=wv_c[ec][:],
                             start=(ec == 0), stop=(ec == NEC - 1))
        nc.vector.tensor_copy(v_t[tt][:, :, 0:D], psv[:])

    def emit_qk(pt, tg):
        cs = slice(tg * TG, (tg + 1) * TG)
        pcol = slice(pt * P, (pt + 1) * P)
        psq = ps.tile([P, TG], f32, tag="mm", name="psq", bufs=2)
        for ec in range(NEC):
            nc.tensor.matmul(psq[:], lhsT=wq_c[ec][:, pcol],
                             rhs=xr[ec][:, cs],
                             start=(ec == 0), stop=(ec == NEC - 1))
        nc.vector.tensor_scalar_add(qT_t[pt][:, cs], psq[:],
                                    bias_t[:, pt:pt + 1])
        psk = ps.tile([P, TG], f32, tag="mm", name="psk", bufs=2)
        for ec in range(NEC):
            nc.tensor.matmul(psk[:], lhsT=wk_c[ec][:, pcol],
                             rhs=xr[ec][:, cs],
                             start=(ec == 0), stop=(ec == NEC - 1))
        nc.vector.tensor_scalar_add(kT_t[pt][:, cs], psk[:],
                                    bias_t[:, NPAIR + pt:NPAIR + pt + 1])

    def emit_proj(tt):
        ts_ = slice(tt * P, (tt + 1) * P)
        for ng in range(E // TG):
            pp = ps.tile([P, TG], f32, tag="mm", name="pp", bufs=2)
            for c in range(NPAIR):
                nc.tensor.matmul(pp[:], lhsT=yT_t[c][:, ts_],
                                 rhs=wp_c[c][:, ng * TG:(ng + 1) * TG],
                                 start=(c == 0), stop=(c == NPAIR - 1))
            ot = ot_pool.tile([P, TG], f32, tag="ot", name="ot")
            nc.vector.tensor_copy(ot[:], pp[:])
            cs_ = slice(ng * TG, (ng + 1) * TG)
            nc.sync.dma_start(out=out[tt * P:tt * P + 64, cs_],
                              in_=ot[0:64, :])
            nc.sync.dma_start(out=out[tt * P + 64:(tt + 1) * P, cs_],
                              in_=ot[64:P, :])

    ones_t = singles.tile([P, D], bf16, tag="ones", name="ones")
    nc.vector.memset(ones_t[:], 1.0)

    # ---- attention for one (pair, q-group): both heads row-tiled ---------
    def emit_att(pt, qg, fillers, last=False):
        qb = qg * TG
        bins = _att_bins(qg)
        nbin = len(bins)
        pend = []  # (bin index, pT tiles per head, bin entries)
        blocks_total = sum(len(b) for b in bins)
        blk_idx = [0, 0]   # per-head running PV block index
        po = [ps.tile([P, TG], f32, tag="o", name=f"po{j}", bufs=2)
              for j in range(2)]

        def emit_pv(ent, pTs):
            for j in range(2):
                for (kt, c0, s0, n) in ent:
                    i = blk_idx[j]
                    nc.tensor.matmul(po[j][:, s0:TG],
                                     lhsT=v_t[kt][:, 2 * pt + j, :],
                                     rhs=pTs[j][:, c0:c0 + n],
                                     start=(i == 0),
                                     stop=(i == blocks_total - 1))
                    blk_idx[j] += 1

        fill_iter = iter(fillers)
        for bi, ent in enumerate(bins):
            width = sum(n for (_, _, _, n) in ent)
            diag = qg > 0 and bi >= nbin - 2
            pss = [ps.tile([P, 2 * TG], f32, tag="s", name=f"pss{j}", bufs=2)
                   for j in range(2)]
            pTs = [pT_pool.tile([P, 2 * TG], bf16, tag="pT", name=f"pT{j}")
                   for j in range(2)]
            # S^T row-tiled: head j on array rows 64j..64j+63.
            for (kt, c0, s0, n) in ent:
                ks = slice(kt * P, (kt + 1) * P)
                qs = slice(qb + s0, qb + TG)
                for j in range(2):
                    rw = slice(64 * j, 64 * (j + 1))
                    nc.tensor.matmul(pss[j][:, c0:c0 + n],
                                     lhsT=kT_t[pt][rw, ks],
                                     rhs=qT_t[pt][rw, qs],
                                     start=True, stop=True)
            for j in range(2):
                nc.scalar.activation(pTs[j][:, 0:width], pss[j][:, 0:width],
                                     Exp, scale=0.125)
                if diag:
                    moff = 0 if bi == nbin - 2 else 896
                    nc.vector.tensor_mul(pTs[j][:, 0:width],
                                         pTs[j][:, 0:width],
                                         mask_t[:, moff:moff + width])
            pend.append((pTs, ent))
            # PV lags S by one bin so the PE isn't waiting on exp.
            if len(pend) > 1:
                ppTs, pent = pend.pop(0)
                emit_pv(pent, ppTs)
            for f in fill_iter:
                f()
                break
        ppTs, pent = pend.pop(0)
        emit_pv(pent, ppTs)
        for f in fill_iter:
            f()

        # ---- normalization (off the PE path) ----
        on = on_pool.tile([D, 2, TG], bf16, tag="on", name="on")
        rc = bc_pool.tile([D, 2, TG], f32, tag="rc", name="rc")
        if last:
            # Short-latency tail path: broadcast the denominator row across
            # partitions with a K=1 PE matmul (PSUM is free by now), then
            # normalize straight out of PSUM.
            den_b = ob_pool.tile([D + 1, 2, TG], bf16, tag="db", name="db")
            for j in range(2):
                nc.vector.tensor_copy(den_b[D:D + 1, j, :],
                                      po[j][D:D + 1, :])
            psb = ps.tile([P, 2 * TG], f32, tag="s", name="psb", bufs=2)
            for j in range(2):
                nc.tensor.matmul(psb[0:D, j * TG:(j + 1) * TG],
                                 lhsT=ones_t[64:65, :],
                                 rhs=den_b[D:D + 1, j, :],
                                 start=True, stop=True)
            for j in range(2):
                nc.vector.reciprocal_approx_fast(
                    out=rc[:, j, :], in_=psb[0:D, j * TG:(j + 1) * TG])
                nc.vector.tensor_mul(on[:, j, :], po[j][0:D, :], rc[:, j, :])
        else:
            ob = ob_pool.tile([D + 1, 2, TG], f32, tag="ob", name="ob")
            for j in range(2):
                nc.vector.tensor_copy(ob[:, j, :], po[j][0:D + 1, :])
            den_d = dr_pool.tile([1, 2, TG], f32, tag="den", name="den")
            nc.gpsimd.dma_start(out=den_d[:], in_=ob[D:D + 1, :, :])
            bcast_in = bass.AP(
                tensor=den_d.tensor, offset=den_d.offset,
                ap=[[0, D]] + [list(a) for a in den_d.ap[1:]])
            bc = bc_pool.tile([D, 2, TG], f32, tag="bc", name="bc")
            nc.gpsimd.dma_start(out=bc[:], in_=bcast_in)
            nc.vector.reciprocal_approx_fast(out=rc[:], in_=bc[:])
            nc.gpsimd.tensor_mul(on[:], ob[0:D, :, :], rc[:])
        for j in range(2):
            nc.gpsimd.dma_start(
                out=yT_t[pt][64 * j:64 * (j + 1), qb:qb + TG],
                in_=on[:, j, :])

    # ---- schedule --------------------------------------------------------
    # Warm-up: V tiles 0-3 and pair-0 tg-0 QK, then attention with
    # remaining projection work interleaved as PE fillers.
    for tt in range(4):
        emit_v(tt)
    emit_qk(0, 0)
    # wp is first needed by out-proj at ~240us; load it after the startup
    # crunch so it doesn't steal early DMA bandwidth from x.
    for c in range(NPAIR):
        nc.sync.dma_start(out=wp_c[c][:], in_=wp[c * P:(c + 1) * P, :])

    fillers = {
        (0, 0): [lambda: emit_qk(0, 1)] +
                [lambda t=t: emit_v(t) for t in range(4, 8)],
        (0, 1): [lambda: emit_qk(0, 2)] +
                [lambda t=t: emit_v(t) for t in range(8, 12)],
        (0, 2): [lambda: emit_qk(0, 3)] +
                [lambda t=t: emit_v(t) for t in range(12, 16)],
        (0, 3): [lambda: emit_qk(1, 0), lambda: emit_qk(1, 1)],
        (1, 0): [lambda: emit_qk(1, 2)],
        (1, 1): [lambda: emit_qk(1, 3)],
        (1, 2): [lambda: emit_qk(2, 0)],
        (1, 3): [lambda: emit_qk(2, 1)],
        (2, 0): [lambda: emit_qk(2, 2), lambda: emit_qk(3, 0)],
        (2, 1): [lambda: emit_qk(2, 3), lambda: emit_qk(3, 1)],
        (2, 2): [lambda: emit_qk(3, 2)],
        (2, 3): [lambda: emit_qk(3, 3)],
        (3, 2): [lambda t=t: emit_proj(t) for t in range(12, 16)],
        (3, 1): [lambda t=t: emit_proj(t) for t in range(8, 12)],
        (3, 0): [lambda t=t: emit_proj(t) for t in range(4, 8)],
    }
    # Pair 3 runs q-groups descending so each completed q-group's output
    # projections fill the next (smaller) attention unit; the tail is only
    # qg 0's projections.
    qg_order = {3: [3, 2, 1, 0]}
    for pt in range(NPAIR):
        for qg in qg_order.get(pt, range(NTG)):
            emit_att(pt, qg, fillers.get((pt, qg), []), last=(pt == 3))
    for tt in range(0, 4):
        emit_proj(tt)

    close_pool(res_pool)
    close_pool(dr_pool)
    close_pool(ot_pool)
    close_pool(on_pool)
    close_pool(bc_pool)
    close_pool(ob_pool)
    close_pool(pT_pool)
    close_pool(ps)
    close_pool(singles)


def _get_program():
    if "nc" not in _CACHE:
        _CACHE["nc"] = _build_program()
    return _CACHE["nc"]


def make_in_maps(x, W_qkv, b_qkv, W_proj):
    """Per-core input dicts: core c -> (batch c%4, head-group c//4)."""
    import ml_dtypes
    x = np.asarray(x, np.float32)
    W_qkv = np.asarray(W_qkv, np.float32)
    b_qkv = np.asarray(b_qkv, np.float32)
    # Packed diagonal-bin mask: segments tri(512)|tri(384)|tri(256)|tri(128);
    # tri(n)[p, j] = (j >= p) for j in [0, n).
    segs = [512, 384, 256, 128]
    binmask = np.zeros((P, sum(segs)), np.float32)
    off = 0
    for n in segs:
        binmask[:, off:off + n] = (np.arange(n)[None, :] >=
                                   np.arange(P)[:, None])
        off += n
    cvt = lambda a: np.ascontiguousarray(a).astype(ml_dtypes.bfloat16)
    in_maps = []
    for c in range(NCORES):
        b, g = c % B, c // B
        gs = slice(g * ESL, (g + 1) * ESL)
        bqs = b_qkv[0 * E:1 * E][gs]
        bks = b_qkv[1 * E:2 * E][gs]
        bias = np.zeros((P, 2 * NPAIR), np.float32)
        for pt in range(NPAIR):
            bias[:, pt] = bqs[pt * P:(pt + 1) * P]
            bias[:, NPAIR + pt] = bks[pt * P:(pt + 1) * P]
        in_maps.append({
            "xT": cvt(x[b].T),
            "wq": cvt(W_qkv[:, 0 * E:1 * E][:, gs]),
            "wk": cvt(W_qkv[:, 1 * E:2 * E][:, gs]),
            "wv": cvt(W_qkv[:, 2 * E:3 * E][:, gs]),
            "wp": cvt(np.asarray(W_proj, np.float32)[gs, :]),
            "bias": np.ascontiguousarray(bias),
            "binmask": cvt(binmask),
        })
    return in_maps


def gather_output(results, b_qkv, b_proj, W_proj):
    """Sum the two row-parallel partials per batch; fold v/proj biases."""
    b_qkv = np.asarray(b_qkv, np.float64)
    W_proj = np.asarray(W_proj, np.float64)
    b_v = b_qkv[2 * E:3 * E]
    const = b_v @ W_proj + np.asarray(b_proj, np.float64)
    out = np.empty((B, T, E), np.float32)
    for b in range(B):
        out[b] = (results[b]["out"].astype(np.float64) +
                  results[b + B]["out"].astype(np.float64) +
                  const).astype(np.float32)
    return out


def run_on_hw(inputs, trace=False, **kwargs):
    from concourse.bass_utils import run_bass_kernel_spmd
    nc = _get_program()
    in_maps = make_in_maps(inputs["x"], inputs["W_qkv"], inputs["b_qkv"],
                           inputs["W_proj"])
    res = run_bass_kernel_spmd(nc, in_maps, list(range(NCORES)), trace=trace,
                               **kwargs)
    out = gather_output(res.results, inputs["b_qkv"], inputs["b_proj"],
                        inputs["W_proj"])
    return out, res


def kernel(x, W_qkv, b_qkv, W_proj, b_proj):
    out, _ = run_on_hw({"x": x, "W_qkv": W_qkv, "b_qkv": b_qkv,
                        "W_proj": W_proj, "b_proj": b_proj})
    return out


# revision 51
# speedup vs baseline: 1.0419x; 1.0405x over previous
"""Trainium2 Bass kernel for nn_CausalSelfAttention_6442450944521.

Sparse-attention causal self-attention block:
  B=4, T=2048 (rows<512: full attention over cols<512; rows>=512: causal),
  E=1024, H=16, D=64.

Sharding: batch (4) x head-group (2 groups of 8 heads) across 8 cores.
Each core computes qkv^T projections, block-sparse attention via S^T = K Q^T
tiles, and its row-slice of the output projection; the two head-group
partials per batch are summed on the host (row-parallel tensor parallelism).

Structure (measured ~315us vs the 354us padded-matmul baseline on the
same hardware):
  - S^T matmuls are ROW-TILED: the two heads of a pair run concurrently on
    the 128x128 PE array (head A rows 0-63 via tile_position (0,0), head B
    rows 64-127 via (64,0)) since the contraction dim is only D=64. kT/qT
    are stored pair-stacked [128, T] so base-partition slicing infers the
    tile positions; the paired matmul's slice shows ~4ns in traces (true
    concurrency), halving S wall time. No zero padding or memsets needed.
  - PV uses a full 128-col stationary [V|ones|zeros] (FWL-eligible; the
    ones column yields the softmax denominator for free); pad fills run
    during the input-DMA dead time at program start.
  - S blocks land in [128, 1024] two-bank PSUM bins; ONE exp per bin per
    head on the scalar engine (320 -> 160 ACT ops; per-op ACT overhead was
    pacing the attention phase).
  - Diagonal-block masks apply via one host-packed [128, 1280] mask tile.
  - Q/K bias adds + PSUM evacuations run on the vector engine; softmax
    normalization multiplies on gpsimd; denominator row -> DRAM ->
    partition-broadcast (gpsimd DMA queue) for pairs 0-2, and a K=1 PE
    broadcast matmul short-latency path for pair 3 (the endgame).
  - Projection (V/QK/out-proj) matmul units are interleaved between
    attention bins in PE program order so the PE fills scalar-exp waits;
    the placement is tuned so no region starves (PE micro-idle lets the
    HAM clock gate re-throttle the PE to 1.2 GHz -- cold 427ns matmuls).
  - Pair 3 runs q-groups descending so each finished q-group's output
    projections fill the next unit; tail is only qg0's projections.
  - Input-load dispatch (~0.6us per dma_start per sequencer) is spread
    across sync/scalar/gpsimd queues; only what the first ~25us needs is
    loaded early (x ch0-1, wv, low halves of wq/wk), wp deferred.
"""

import os
import sys

if "/opt/trn_rl_repo" not in sys.path:
    sys.path.insert(0, "/opt/trn_rl_repo")

import numpy as np

# Problem constants (hardcoded per harness contract).
B = 4
T = 2048
E = 1024
H = 16
D = 64
NCORES = 8
HPC = H // 2          # heads per core = 8
ESL = HPC * D         # per-core E-slice = 512
P = 128               # SBUF/PSUM partitions
TG = 512              # q-group width
NTG = T // TG         # 4
NTT = T // P          # 16
NEC = E // P          # 8 contraction chunks over E
NPAIR = HPC // 2      # 4 head-pair tiles

_CACHE = {}


def _att_bins(qg):
    """Bins of S^T blocks for q-group qg. Each bin is a list of
    (kt, c0, s0, n): k-tile index, column offset in the [128,1024] bin,
    q-offset within the group, and width. Total bin width <= 1024."""
    bins = []
    nf = 4 if qg == 0 else 4 * qg
    for k0 in range(0, nf, 2):
        bins.append([(k0, 0, 0, TG), (k0 + 1, TG, 0, TG)])
    if qg > 0:
        m0 = 4 * qg
        bins.append([(m0, 0, 0, 512), (m0 + 1, 512, 128, 384)])
        bins.append([(m0 + 2, 0, 256, 256), (m0 + 3, 256, 384, 128)])
    return bins


def _build_program():
    import concourse.bass as bass
    import concourse.tile as tile
    from concourse import bacc, mybir

    f32 = mybir.dt.float32
    bf16 = mybir.dt.bfloat16

    nc = bacc.Bacc("TRN2", target_bir_lowering=False, debug=False,
                   num_devices=NCORES)

    xT = nc.dram_tensor("xT", [E, T], bf16, kind="ExternalInput").ap()
    wq = nc.dram_tensor("wq", [E, ESL], bf16, kind="ExternalInput").ap()
    wk = nc.dram_tensor("wk", [E, ESL], bf16, kind="ExternalInput").ap()
    wv = nc.dram_tensor("wv", [E, ESL], bf16, kind="ExternalInput").ap()
    wp = nc.dram_tensor("wp", [ESL, E], bf16, kind="ExternalInput").ap()
    bias = nc.dram_tensor("bias", [P, 2 * NPAIR], f32,
                          kind="ExternalInput").ap()
    binmask = nc.dram_tensor("binmask", [P, 1280], bf16,
                             kind="ExternalInput").ap()
    out = nc.dram_tensor("out", [T, E], f32, kind="ExternalOutput").ap()

    with tile.TileContext(nc) as tc:
        _body(nc, tc, tile, mybir, bass,
              xT, wq, wk, wv, wp, bias, binmask, out)

    nc.compile()
    return nc


def _body(nc, tc, tile, mybir, bass,
          xT, wq, wk, wv, wp, bias, binmask, out):
    f32 = mybir.dt.float32
    bf16 = mybir.dt.bfloat16
    Exp = mybir.ActivationFunctionType.Exp

    cms = {}

    def open_pool(name, bufs, space=None, side=None):
        kw = {}
        if space:
            kw["space"] = space
        if side:
            kw["side"] = side
        cm = tc.tile_pool(name=name, bufs=bufs, **kw)
        pool = cm.__enter__()
        cms[id(pool)] = cm
        return pool

    def close_pool(pool):
        cms.pop(id(pool)).__exit__(None, None, None)

    # ---- pools ----------------------------------------------------------
    singles = open_pool("singles", 1)
    ps = open_pool("ps", 2, space="PSUM")
    pT_pool = open_pool("pT", 5)
    ob_pool = open_pool("ob", 2)
    bc_pool = open_pool("bc", 2)
    on_pool = open_pool("on", 2)
    ot_pool = open_pool("ot", 4)
    dr_pool = open_pool("dr", 2, space="DRAM")
    # right-stack: big resident tensors
    res_pool = open_pool("res", 1, side="right")

    # ---- resident loads --------------------------------------------------
    mask_t = singles.tile([P, 1280], bf16, tag="mask", name="mask")
    nc.sync.dma_start(out=mask_t[:], in_=binmask)
    bias_t = singles.tile([P, 2 * NPAIR], f32, tag="bias", name="bias")
    nc.sync.dma_start(out=bias_t[:], in_=bias)

    # Input loads: dispatch is the startup bottleneck (~0.6us per dma_start
    # on one sequencer), so spread it across the three DMA-capable engines.
    # sync: x chunk 0 + wk + x chunk 1 + wp; scalar: wv + wq (idle until the
    # first exp anyway); gpsimd: x chunks 2-3.
    xr = [res_pool.tile([P, T], bf16, tag=f"xr{ec}", name=f"xr{ec}")
          for ec in range(NEC)]
    wq_c, wk_c, wv_c, wp_c = [], [], [], []
    for ec in range(NEC):
        wv_c.append(res_pool.tile([P, ESL], bf16, tag="wv", name="wvc",
                                  bufs=NEC))
        wq_c.append(res_pool.tile([P, ESL], bf16, tag="wq", name="wqc",
                                  bufs=NEC))
        wk_c.append(res_pool.tile([P, ESL], bf16, tag="wk", name="wkc",
                                  bufs=NEC))
    H2 = ESL // 2
    for ec in range(NEC):
        nc.sync.dma_start(out=xr[ec][:, 0:T // 4],
                          in_=xT[ec * P:(ec + 1) * P, 0:T // 4])
        nc.scalar.dma_start(out=wv_c[ec][:], in_=wv[ec * P:(ec + 1) * P, :])
        nc.gpsimd.dma_start(out=wk_c[ec][:, 0:H2],
                            in_=wk[ec * P:(ec + 1) * P, 0:H2])
    for ec in range(NEC):
        nc.sync.dma_start(out=xr[ec][:, T // 4:T // 2],
                          in_=xT[ec * P:(ec + 1) * P, T // 4:T // 2])
        nc.scalar.dma_start(out=wq_c[ec][:, 0:H2],
                            in_=wq[ec * P:(ec + 1) * P, 0:H2])
        nc.gpsimd.dma_start(out=xr[ec][:, T // 2:3 * T // 4],
                            in_=xT[ec * P:(ec + 1) * P, T // 2:3 * T // 4])
    for ec in range(NEC):
        nc.gpsimd.dma_start(out=xr[ec][:, 3 * T // 4:T],
                            in_=xT[ec * P:(ec + 1) * P, 3 * T // 4:T])
        nc.scalar.dma_start(out=wq_c[ec][:, H2:ESL],
                            in_=wq[ec * P:(ec + 1) * P, H2:ESL])
        nc.sync.dma_start(out=wk_c[ec][:, H2:ESL],
                          in_=wk[ec * P:(ec + 1) * P, H2:ESL])
    wp_c = [res_pool.tile([P, E], bf16, tag="wp", name="wpc", bufs=NPAIR)
            for c in range(NPAIR)]

    qT_t = [res_pool.tile([P, T], bf16, tag=f"qT{i}", name=f"qT{i}")
            for i in range(NPAIR)]
    kT_t = [res_pool.tile([P, T], bf16, tag=f"kT{i}", name=f"kT{i}")
            for i in range(NPAIR)]
    yT_t = [res_pool.tile([P, T], bf16, tag=f"yT{i}", name=f"yT{i}")
            for i in range(NPAIR)]
    # V per T-tile: per head [V(64) | ones | zeros(63)] = full 128-col
    # stationary (FWL-eligible). Zero/ones fills run during the input-DMA
    # dead time at program start.
    v_t = [res_pool.tile([P, HPC, P], bf16, tag=f"v{i}", name=f"v{i}")
           for i in range(NTT)]
    for tt in range(NTT):
        nc.vector.memset(v_t[tt][:, :, D + 1:], 0.0)
        nc.vector.memset(v_t[tt][:, :, D:D + 1], 1.0)

    # ---- filler units (PE work interleaved between attention bins) ------
    def emit_v(tt):
        ts_ = slice(tt * P, (tt + 1) * P)
        psv = ps.tile([P, ESL], f32, tag="mm", name="psv", bufs=2)
        for ec in range(NEC):
            nc.tensor.matmul(psv[:], lhsT=xr[ec][:, ts_], rhs=wv_c[ec][:],
                             start=(ec == 0), stop=(ec == NEC - 1))
        nc.vector.tensor_copy(v_t[tt][:, :, 0:D], psv[:])

    def emit_qk(pt, tg):
        cs = slice(tg * TG, (tg + 1) * TG)
        pcol = slice(pt * P, (pt + 1) * P)
        psq = ps.tile([P, TG], f32, tag="mm", name="psq", bufs=2)
        for ec in range(NEC):
            nc.tensor.matmul(psq[:], lhsT=wq_c[ec][:, pcol],
                             rhs=xr[ec][:, cs],
                             start=(ec == 0), stop=(ec == NEC - 1))
        nc.vector.tensor_scalar_add(qT_t[pt][:, cs], psq[:],
                                    bias_t[:, pt:pt + 1])
        psk = ps.tile([P, TG], f32, tag="mm", name="psk", bufs=2)
        for ec in range(NEC):
            nc.tensor.matmul(psk[:], lhsT=wk_c[ec][:, pcol],
                             rhs=xr[ec][:, cs],
                             start=(ec == 0), stop=(ec == NEC - 1))
        nc.vector.tensor_scalar_add(kT_t[pt][:, cs], psk[:],
                                    bias_t[:, NPAIR + pt:NPAIR + pt + 1])

    def emit_proj(tt):
        ts_ = slice(tt * P, (tt + 1) * P)
        for ng in range(E // TG):
            pp = ps.tile([P, TG], f32, tag="mm", name="pp", bufs=2)
            for c in range(NPAIR):
                nc.tensor.matmul(pp[:], lhsT=yT_t[c][:, ts_],
                                 rhs=wp_c[c][:, ng * TG:(ng + 1) * TG],
                                 start=(c == 0), stop=(c == NPAIR - 1))
            ot = ot_pool.tile([P, TG], f32, tag="ot", name="ot")
            nc.vector.tensor_copy(ot[:], pp[:])
            cs_ = slice(ng * TG, (ng + 1) * TG)
            nc.sync.dma_start(out=out[tt * P:tt * P + 64, cs_],
                              in_=ot[0:64, :])
            nc.sync.dma_start(out=out[tt * P + 64:(tt + 1) * P, cs_],
                              in_=ot[64:P, :])

    ones_t = singles.tile([P, D], bf16, tag="ones", name="ones")
    nc.vector.memset(ones_t[:], 1.0)

    # ---- attention for one (pair, q-group): both heads row-tiled ---------
    def emit_att(pt, qg, fillers, last=False):
        qb = qg * TG
        bins = _att_bins(qg)
        nbin = len(bins)
        pend = []  # (bin index, pT tiles per head, bin entries)
        blocks_total = sum(len(b) for b in bins)
        blk_idx = [0, 0]   # per-head running PV block index
        po = [ps.tile([P, TG], f32, tag="o", name=f"po{j}", bufs=2)
              for j in range(2)]

        def emit_pv(ent, pTs):
            for j in range(2):
                for (kt, c0, s0, n) in ent:
                    i = blk_idx[j]
                    nc.tensor.matmul(po[j][:, s0:TG],
                                     lhsT=v_t[kt][:, 2 * pt + j, :],
                                     rhs=pTs[j][:, c0:c0 + n],
                                     start=(i == 0),
                                     stop=(i == blocks_total - 1))
                    blk_idx[j] += 1

        fill_iter = iter(fillers)
        for bi, ent in enumerate(bins):
            width = sum(n for (_, _, _, n) in ent)
            diag = qg > 0 and bi >= nbin - 2
            pss = [ps.tile([P, 2 * TG], f32, tag="s", name=f"pss{j}", bufs=2)
                   for j in range(2)]
            pTs = [pT_pool.tile([P, 2 * TG], bf16, tag="pT", name=f"pT{j}")
                   for j in range(2)]
            # S^T row-tiled: head j on array rows 64j..64j+63.
            for (kt, c0, s0, n) in ent:
                ks = slice(kt * P, (kt + 1) * P)
                qs = slice(qb + s0, qb + TG)
                for j in range(2):
                    rw = slice(64 * j, 64 * (j + 1))
                    nc.tensor.matmul(pss[j][:, c0:c0 + n],
                                     lhsT=kT_t[pt][rw, ks],
                                     rhs=qT_t[pt][rw, qs],
                                     start=True, stop=True)
            for j in range(2):
                nc.scalar.activation(pTs[j][:, 0:width], pss[j][:, 0:width],
                                     Exp, scale=0.125)
                if diag:
                    moff = 0 if bi == nbin - 2 else 896
                    nc.vector.tensor_mul(pTs[j][:, 0:width],
                                         pTs[j][:, 0:width],
                                         mask_t[:, moff:moff + width])
            pend.append((pTs, ent))
            # PV lags S by one bin so the PE isn't waiting on exp.
            if len(pend) > 1:
                ppTs, pent = pend.pop(0)
                emit_pv(pent, ppTs)
            for f in fill_iter:
                f()
                break
        ppTs, pent = pend.pop(0)
        emit_pv(pent, ppTs)
        for f in fill_iter:
            f()

        # ---- normalization (off the PE path) ----
        on = on_pool.tile([D, 2, TG], bf16, tag="on", name="on")
        rc = bc_pool.tile([D, 2, TG], f32, tag="rc", name="rc")
        if last:
            # Short-latency tail path: broadcast the denominator row across
            # partitions with a K=1 PE matmul (PSUM is free by now), then
            # normalize straight out of PSUM.
            den_b = ob_pool.tile([D + 1, 2, TG], bf16, tag="db", name="db")
            for j in range(2):
                nc.vector.tensor_copy(den_b[D:D + 1, j, :],
                                      po[j][D:D + 1, :])
            psb = ps.tile([P, 2 * TG], f32, tag="s", name="psb", bufs=2)
            for j in range(2):
                nc.tensor.matmul(psb[0:D, j * TG:(j + 1) * TG],
                                 lhsT=ones_t[64:65, :],
                                 rhs=den_b[D:D + 1, j, :],
                                 start=True, stop=True)
            for j in range(2):
                nc.vector.reciprocal_approx_fast(
                    out=rc[:, j, :], in_=psb[0:D, j * TG:(j + 1) * TG])
                nc.vector.tensor_mul(on[:, j, :], po[j][0:D, :], rc[:, j, :])
        else:
            ob = ob_pool.tile([D + 1, 2, TG], f32, tag="ob", name="ob")
            for j in range(2):
                nc.vector.tensor_copy(ob[:, j, :], po[j][0:D + 1, :])
            den_d = dr_pool.tile([1, 2, TG], f32, tag="den", name="den")
            nc.gpsimd.dma_start(out=den_d[:], in_=ob[D:D + 1, :, :])
            bcast_in = bass.AP(
                tensor=den_d.tensor, offset=den_d.offset,
                ap=[[0, D]] + [list(a) for a in den_d.ap[1:]])
            bc = bc_pool.tile([D, 2, TG], f32, tag="bc", name="bc")
            nc.gpsimd.dma_start(out=bc[:], in_=bcast_in)
            nc.vector.reciprocal_approx_fast(out=rc[:], in_=bc[:])
            nc.gpsimd.tensor_mul(on[:], ob[0:D, :, :], rc[:])
        for j in range(2):
            nc.gpsimd.dma_start(
                out=yT_t[pt][64 * j:64 * (j + 1), qb:qb + TG],
                in_=on[:, j, :])

    # ---- schedule --------------------------------------------------------
    # Warm-up: V tiles 0-3 and pair-0 tg-0 QK, then attention with
    # remaining projection work interleaved as PE fillers.
    for tt in range(4):
        emit_v(tt)
    emit_qk(0, 0)
    # wp is first needed by out-proj at ~240us; load it after the startup
    # crunch so it doesn't steal early DMA bandwidth from x.
    for c in range(NPAIR):
        nc.sync.dma_start(out=wp_c[c][:], in_=wp[c * P:(c + 1) * P, :])

    fillers = {
        (0, 0): [lambda: emit_qk(0, 1)] +
                [lambda t=t: emit_v(t) for t in range(4, 8)],
        (0, 1): [lambda: emit_qk(0, 2)] +
                [lambda t=t: emit_v(t) for t in range(8, 12)],
        (0, 2): [lambda: emit_qk(0, 3)] +
                [lambda t=t: emit_v(t) for t in range(12, 16)],
        (0, 3): [lambda: emit_qk(1, 0), lambda: emit_qk(1, 1)],
        (1, 0): [lambda: emit_qk(1, 2)],
        (1, 1): [lambda: emit_qk(1, 3)],
        (1, 2): [lambda: emit_qk(2, 0)],
        (1, 3): [lambda: emit_qk(2, 1)],
        (2, 0): [lambda: emit_qk(2, 2), lambda: emit_qk(3, 0)],
        (2, 1): [lambda: emit_qk(2, 3), lambda: emit_qk(3, 1)],
        (2, 2): [lambda: emit_qk(3, 2)],
        (2, 3): [lambda: emit_qk(3, 3)],
        (3, 2): [lambda t=t: emit_proj(t) for t in range(12, 16)],
        (3, 1): [lambda t=t: emit_proj(t) for t in range(8, 12)],
        (3, 0): [lambda t=t: emit_proj(t) for t in range(4, 8)],
    }
    # Pair 3 runs q-groups descending so each completed q-group's output
    # projections fill the next (smaller) attention unit; the tail is only
    # qg 0's projections.
    qg_order = {3: [3, 2, 1, 0]}
    for pt in range(NPAIR):
        for qg in qg_order.get(pt, range(NTG)):
            emit_att(pt, qg, fillers.get((pt, qg), []), last=(pt == 3))
    for tt in range(0, 4):
        emit_proj(tt)

    close_pool(res_pool)
    close_pool(dr_pool)
    close_pool(ot_pool)
    close_pool(on_pool)
    close_pool(bc_pool)
    close_pool(ob_pool)
    close_pool(pT_pool)
    close_pool(ps)
    close_pool(singles)


def _get_program():
    if "nc" not in _CACHE:
        _CACHE["nc"] = _build_program()
    return _CACHE["nc"]


def make_in_maps(x, W_qkv, b_qkv, W_proj):
    """Per-core input dicts: core c -> (batch c%4, head-group c//4)."""
    import ml_dtypes
    x = np.asarray(x, np.float32)
    W_qkv = np.asarray(W_qkv, np.float32)
    b_qkv = np.asarray(b_qkv, np.float32)
    # Packed diagonal-bin mask: segments tri(512)|tri(384)|tri(256)|tri(128);
    # tri(n)[p, j] = (j >= p) for j in [0, n).
    segs = [512, 384, 256, 128]
    binmask = np.zeros((P, sum(segs)), np.float32)
    off = 0
    for n in segs:
        binmask[:, off:off + n] = (np.arange(n)[None, :] >=
                                   np.arange(P)[:, None])
        off += n
    cvt = lambda a: np.ascontiguousarray(a).astype(ml_dtypes.bfloat16)
    in_maps = []
    for c in range(NCORES):
        b, g = c % B, c // B
        gs = slice(g * ESL, (g + 1) * ESL)
        bqs = b_qkv[0 * E:1 * E][gs]
        bks = b_qkv[1 * E:2 * E][gs]
        bias = np.zeros((P, 2 * NPAIR), np.float32)
        for pt in range(NPAIR):
            bias[:, pt] = bqs[pt * P:(pt + 1) * P]
            bias[:, NPAIR + pt] = bks[pt * P:(pt + 1) * P]
        in_maps.append({
            "xT": cvt(x[b].T),
            "wq": cvt(W_qkv[:, 0 * E:1 * E][:, gs]),
            "wk": cvt(W_qkv[:, 1 * E:2 * E][:, gs]),
            "wv": cvt(W_qkv[:, 2 * E:3 * E][:, gs]),
            "wp": cvt(np.asarray(W_proj, np.float32)[gs, :]),
            "bias": np.ascontiguousarray(bias),
            "binmask": cvt(binmask),
        })
    return in_maps


def gather_output(results, b_qkv, b_proj, W_proj):
    """Sum the two row-parallel partials per batch; fold v/proj biases."""
    b_qkv = np.asarray(b_qkv, np.float64)
    W_proj = np.asarray(W_proj, np.float64)
    b_v = b_qkv[2 * E:3 * E]
    const = b_v @ W_proj + np.asarray(b_proj, np.float64)
    out = np.empty((B, T, E), np.float32)
    for b in range(B):
        out[b] = (results[b]["out"].astype(np.float64) +
                  results[b + B]["out"].astype(np.float64) +
                  const).astype(np.float32)
    return out


def run_on_hw(inputs, trace=False, **kwargs):
    from concourse.bass_utils import run_bass_kernel_spmd
    nc = _get_program()
    in_maps = make_in_maps(inputs["x"], inputs["W_qkv"], inputs["b_qkv"],
                           inputs["W_proj"])
    res = run_bass_kernel_spmd(nc, in_maps, list(range(NCORES)), trace=trace,
                               **kwargs)
    out = gather_output(res.results, inputs["b_qkv"], inputs["b_proj"],
                        inputs["W_proj"])
    return out, res


def kernel(x, W_qkv, b_qkv, W_proj, b_proj):
    out, _ = run_on_hw({"x": x, "W_qkv": W_qkv, "b_qkv": b_qkv,
                        "W_proj": W_proj, "b_proj": b_proj})
    return out
